# revision 2
# baseline (speedup 1.0000x reference)
"""Cross-modal triplet loss (margin ranking on hardest pos/neg pairs) on 8 trn2 NeuronCores.

Strategy (per sharding hint): shard rows of modal1 across the 8 cores (512 rows
each); replicate modal2 and targets. Each core computes its 512x4096 slab of

    psum[m, j] = dot(m1[m], m2[j]) - sq2[j]/2 - (BIG/2) * mask[m, j]

entirely on the PE array: the main dot-product runs as fp8(e4m3) matmuls in
DoubleRow perf mode (two 128-row k-tiles per instruction), and the
sq2/same-identity-mask terms ride along as one small bf16 "aug" matmul per
accumulation group (the mask is rank-64 over the 64 ids: -8192*onehot1 x
onehot2; -sq2/2 enters as a bf16 hi/lo pair for ~16-bit accuracy).

Row-wise min(psum) then locates the hardest positive (same-id entries sit
-8192 below all diff-id entries) and max(psum) the hardest negative, exactly
as in the fp32 formulation:

    ap^2 = sq1[m] - 2*min_j(psum) - BIG      (hardest-positive distance^2)
    an^2 = sq1[m] - 2*max_j(psum)            (hardest-negative distance^2)

All operand layout work (k-major transposes, fp8/bf16 casts, one-hot mask
construction, sq1/sq2 row norms) happens on the host while preparing the
shard buffers, so the device program is nothing but DMA-in -> matmul ->
row-reduce -> DMA-out. Each weight tile is streamed against all 8 PSUM banks
(one per 512-column output chunk) so LDWEIGHTS cost amortizes 8x. The per-row
psum min/max (512 rows x 2 values per core) return to the host, which applies
the sq1 shift, sqrt, margin hinge, and the mean over all 4096 rows.
"""

import functools

import ml_dtypes
import numpy as np

import concourse.bass as bass
import concourse.mybir as mybir
import concourse.tile as tile
from concourse import bacc
from concourse.bass_utils import run_bass_kernel_spmd

F32 = mybir.dt.float32
BF16 = mybir.dt.bfloat16
F8 = mybir.dt.float8e4
OP = mybir.AluOpType
AX = mybir.AxisListType.X
DR = mybir.MatmulPerfMode.DoubleRow

NP_F8 = ml_dtypes.float8_e4m3
NP_BF16 = ml_dtypes.bfloat16

N, D, NIDS, P = 4096, 2048, 64, 128
NCORES = 8
SH = N // NCORES      # 512 rows of modal1 per core
MT = SH // P          # 4 m-tiles per core
KT = D // P           # 16 k-tiles
KT2 = KT // 2         # 8 double-k-tiles (DoubleRow covers 256 of K each)
CHUNK = 512           # output columns per PSUM bank
NJC = N // CHUNK      # 8 column chunks
KAUG = 66             # 64 one-hot mask rows + sq2 hi/lo
BIG = 16384.0         # separates same-id from diff-id psum values
EPS = 1e-12


def _build() -> bass.Bass:
    nc = bacc.Bacc(num_swdge_queues=4)
    m1d = nc.dram_tensor("m1dr", [P, KT, SH], F8, kind="ExternalInput")
    m2d = nc.dram_tensor("m2dr", [P, KT, N], F8, kind="ExternalInput")
    laugd = nc.dram_tensor("laug", [KAUG, SH], BF16, kind="ExternalInput")
    raugd = nc.dram_tensor("raug", [KAUG, N], BF16, kind="ExternalInput")
    outd = nc.dram_tensor("out", [P, 2 * MT], F32, kind="ExternalOutput")

    with tile.TileContext(nc) as tc:
        with (
            tc.tile_pool(name="const", bufs=1) as const,
            tc.tile_pool(name="ps", bufs=8, space=bass.MemorySpace.PSUM) as psp,
        ):
            m1sb = const.tile([P, KT, SH], F8)
            nc.sync.dma_start(m1sb[:, :, :], m1d[:, :, :])
            laug = const.tile([KAUG, SH], BF16)
            nc.scalar.dma_start(laug[:, :], laugd[:, :])
            raug = const.tile([KAUG, N], BF16)
            nc.scalar.dma_start(raug[:, :], raugd[:, :])

            m2t = []
            for t in range(KT2):
                m2tile = const.tile([P, 2, N], F8, name=f"m2t{t}")
                eng = nc.sync if t % 2 == 0 else nc.scalar
                eng.dma_start(m2tile[:, :, :], m2d[:, 2 * t : 2 * t + 2, :])
                m2t.append(m2tile)

            minb = [
                const.tile([P, NJC], F32, name=f"minb{mt}") for mt in range(MT)
            ]
            maxb = [
                const.tile([P, NJC], F32, name=f"maxb{mt}") for mt in range(MT)
            ]

            for mt in range(MT):
                ms = slice(mt * P, (mt + 1) * P)
                pss = [
                    psp.tile([P, CHUNK], F32, tag="ps", name=f"ps{mt}_{jc}")
                    for jc in range(NJC)
                ]
                for t in range(KT2):
                    lw = m1sb[:, 2 * t : 2 * t + 2, ms]
                    for jc in range(NJC):
                        nc.tensor.matmul(
                            pss[jc][:, :],
                            lw,
                            m2t[t][:, :, jc * CHUNK : (jc + 1) * CHUNK],
                            start=(t == 0),
                            stop=False,
                            perf_mode=DR,
                        )
                for jc in range(NJC):
                    nc.tensor.matmul(
                        pss[jc][:, :],
                        laug[:, ms],
                        raug[:, jc * CHUNK : (jc + 1) * CHUNK],
                        start=False,
                        stop=True,
                    )
                for jc in range(NJC):
                    nc.vector.tensor_reduce(
                        minb[mt][:, jc : jc + 1], pss[jc][:, :], AX, OP.min
                    )
                    nc.vector.tensor_reduce(
                        maxb[mt][:, jc : jc + 1], pss[jc][:, :], AX, OP.max
                    )

            osb = const.tile([P, 2 * MT], F32)
            for mt in range(MT):
                nc.vector.tensor_reduce(
                    osb[:, mt : mt + 1], minb[mt][:, :], AX, OP.min
                )
                nc.vector.tensor_reduce(
                    osb[:, MT + mt : MT + mt + 1], maxb[mt][:, :], AX, OP.max
                )
            nc.sync.dma_start(outd[:, :], osb[:, :])

    nc.finalize()
    return nc


@functools.lru_cache(maxsize=1)
def _get_program() -> bass.Bass:
    return _build()


def _make_in_maps(m1, m2, targets):
    ids = np.arange(NIDS)
    tgt = np.asarray(targets).astype(np.int64).reshape(N)

    # k-major fp8 operand layouts: tile[p, s, x] = X[x, s*128 + p]
    m2dr = np.ascontiguousarray(
        m2.astype(NP_F8).reshape(N, KT, P).transpose(2, 1, 0)
    )

    # right aug features (shared): one-hot ids + sq2 hi/lo (bf16 pair)
    sq2h = -0.5 * np.sum(m2.astype(np.float64) * m2.astype(np.float64), axis=1)
    shi = sq2h.astype(np.float32).astype(NP_BF16)
    slo = (sq2h - shi.astype(np.float64)).astype(np.float32).astype(NP_BF16)
    raug = np.zeros((KAUG, N), dtype=NP_BF16)
    raug[:NIDS] = (tgt[None, :] == ids[:, None]).astype(NP_BF16)
    raug[NIDS] = shi
    raug[NIDS + 1] = slo
    raug = np.ascontiguousarray(raug)

    maps = []
    for c in range(NCORES):
        m1c = m1[c * SH : (c + 1) * SH]
        m1dr = np.ascontiguousarray(
            m1c.astype(NP_F8).reshape(SH, KT, P).transpose(2, 1, 0)
        )
        tgtc = tgt[c * SH : (c + 1) * SH]
        laug = np.zeros((KAUG, SH), dtype=NP_BF16)
        laug[:NIDS] = (-BIG / 2.0) * (tgtc[None, :] == ids[:, None]).astype(
            np.float32
        )
        laug[NIDS] = 1.0
        laug[NIDS + 1] = 1.0
        maps.append(
            {
                "m1dr": m1dr,
                "m2dr": m2dr,
                "laug": np.ascontiguousarray(laug),
                "raug": raug,
            }
        )
    return maps


def run(modal1_inputs, modal2_inputs, targets, margin, trace=False):
    m1 = np.ascontiguousarray(np.asarray(modal1_inputs, dtype=np.float32))
    m2 = np.ascontiguousarray(np.asarray(modal2_inputs, dtype=np.float32))
    nc = _get_program()
    res = run_bass_kernel_spmd(
        nc, _make_in_maps(m1, m2, targets), list(range(NCORES)), trace=trace
    )

    # host finale: undo the psum encoding, sqrt, hinge, mean
    pmin = np.empty(N, dtype=np.float64)
    pmax = np.empty(N, dtype=np.float64)
    for c, r in enumerate(res.results):
        o = np.asarray(r["out"], dtype=np.float64)  # [P, 2*MT]
        pmin[c * SH : (c + 1) * SH] = o[:, :MT].T.reshape(SH)
        pmax[c * SH : (c + 1) * SH] = o[:, MT:].T.reshape(SH)

    sq1 = np.sum(m1.astype(np.float64) * m1.astype(np.float64), axis=1)
    ap2 = np.maximum(sq1 - 2.0 * pmin - BIG, EPS)
    an2 = np.maximum(sq1 - 2.0 * pmax, EPS)
    ap = np.sqrt(ap2)
    an = np.sqrt(an2)
    loss = np.float32(np.mean(np.maximum(ap - an + float(margin), 0.0)))
    prec = np.float32(np.mean(an > ap))
    return (loss, prec), res


def kernel(modal1_inputs, modal2_inputs, targets, margin):
    (loss, prec), _ = run(modal1_inputs, modal2_inputs, targets, margin)
    return loss, prec
